# revision 7
# baseline (speedup 1.0000x reference)
"""Chunked-causal attention (MemoryEfficientAttention) for Trainium2.

Full inputs q,k,v: [2, 16, 2048, 64] fp32. Causal attention per (batch, head);
chunked reference == plain causal attention. 32 (b,h) slices split 4-per-core
across 8 NeuronCores (data/head parallel, no collectives).

Per-core kernel (4 heads, S=2048, D=64), v2 design:
  - q,k loaded natural [128, 16, 64], cast to fp16 into head-PAIR tiles
    [128, 16, 128] (head a in cols 0:64, head b in 64:128), then transposed
    d-major by the DMA XBAR (InstDmaTransposeAnt): its fold semantics
    out[r % 128, r // 128, p] = in[p, r] land head a's qT on partitions 0:64
    and head b's on 64:128 - zero PE cost, no fixup copies. Head b's matmuls
    use base partition 64 (PE tile position (64, 0)).
  - [V|1] cast to fp16 [128, 16, 65] (ones col -> softmax denominator).
  - per query-pass (1024 wide), per key block jb (128 wide):
      scT[j,i] = kT_jb.T @ qT  (fp16 matmuls -> fp32 PSUM, <=512 col chunks)
      eT = exp(scT/8) fp16     (one ACT instr per (pass, jb))
      diagonal block masked in place on GPSIMD (affine_select, fill 0)
      acc[d|1, i] += vext_jb.T @ eT  (fp16, accumulated in [65,1024] PSUM)
  - epilogue per pass: acc -> oT fp16 [80, 8, 128] (DVE; rows 65:80 pad),
    one XBAR transpose -> o_ti [128, 8, 80] i-major, rcp = 1/denom (DVE),
    out = val*rcp -> f32 (DVE), single DMA store per pass.

PSUM: scT [128,1024] x2 bufs (4 banks) + acc [65,1024] x2 bufs (4 banks) = 8.
Softmax without max-subtraction: scores/8 ~ N(0,1), far from fp32 exp range.
"""

import hashlib
import os

import numpy as np

B, H, S, D = 2, 16, 2048, 64
N_CORES = 8
HPC = (B * H) // N_CORES  # heads per core
NB = S // 128             # 128-row key blocks per head
PASS_W = 1024             # query pass width (2 PSUM banks)
CHUNK = 512               # AV accumulator chunk (1 PSUM bank)

_NC = None


def _install_neff_cache():
    """Content-addressed NEFF cache so repeat runs skip the ~2min walrus compile."""
    import concourse.bass2jax as bass2jax

    real_compile = bass2jax.compile_bir_kernel
    if getattr(bass2jax, "_neff_cache_installed", False):
        return
    cache_dir = os.path.expanduser("~/.cache/bass_neff")
    os.makedirs(cache_dir, exist_ok=True)

    def cached_compile(bir_json, tmpdir, neff_name="file.neff"):
        key = hashlib.sha256(bir_json).hexdigest()[:24]
        path = os.path.join(cache_dir, f"{key}.neff")
        if os.path.exists(path):
            dst = os.path.join(tmpdir, neff_name)
            with open(path, "rb") as f_in, open(dst, "wb") as f_out:
                f_out.write(f_in.read())
            return dst
        neff = real_compile(bir_json, tmpdir, neff_name)
        with open(neff, "rb") as f_in, open(path + ".tmp", "wb") as f_out:
            f_out.write(f_in.read())
        os.replace(path + ".tmp", path)
        return neff

    bass2jax.compile_bir_kernel = cached_compile
    bass2jax._neff_cache_installed = True


def _emit_head(nc, mybir, pools, h, qT, kT, vext, o_d):
    """One head's QK/exp/AV/epilogue. qT/kT [64, S] may start at partition 64."""
    f32 = mybir.dt.float32
    f16 = mybir.dt.float16
    Exp = mybir.ActivationFunctionType.Exp
    exps, epi, ps = pools

    for p in range(2):
        ilo_p, ihi_p = p * PASS_W, (p + 1) * PASS_W
        acc = ps.tile([D + 1, PASS_W], f32, tag="acc", name=f"ac{h}{p}")
        n_jb = 8 * p + 8
        for jb in range(n_jb):
            j0 = jb * 128
            i_lo = max(j0, ilo_p)
            w = ihi_p - i_lo
            scT = ps.tile([128, PASS_W], f32, tag="sc", name="scT")
            for c0 in range(0, w, CHUNK):
                c1 = min(c0 + CHUNK, w)
                nc.tensor.matmul(
                    scT[:, c0:c1],
                    kT[:, j0 : j0 + 128],
                    qT[:, i_lo + c0 : i_lo + c1],
                    start=True,
                    stop=True,
                )
            eT = exps.tile([128, PASS_W], f16, tag="eT", name="eT")
            nc.scalar.activation(
                eT[:, 0:w], scT[:, 0:w], Exp, scale=float(D) ** -0.5
            )
            if j0 >= ilo_p:
                # diagonal block: keep j <= i (iota = i - j >= 0)
                nc.gpsimd.affine_select(
                    out=eT[:, 0:128],
                    in_=eT[:, 0:128],
                    compare_op=mybir.AluOpType.is_ge,
                    fill=0.0,
                    pattern=[[1, 128]],
                    channel_multiplier=-1,
                )
            for c in range(2):
                g = 2 * p + c
                ch_lo, ch_hi = ilo_p + c * CHUNK, ilo_p + (c + 1) * CHUNK
                if ch_hi <= i_lo:
                    continue
                a_lo = max(i_lo, ch_lo)
                nc.tensor.matmul(
                    acc[:, a_lo - ilo_p : ch_hi - ilo_p],
                    vext[:, jb, :],
                    eT[:, a_lo - i_lo : ch_hi - i_lo],
                    start=(jb == 0),
                    stop=(jb == 4 * g + 3),
                )

        # ---- epilogue: normalize + transpose via XBAR -------------
        # oT [80, (blk, pp)] fp16; one XBAR instr yields o_ti[pp, blk, d].
        # Rows 65:80 are pad (XBAR wants a multiple of 16 rows).
        oT = epi.tile([80, 8, 128], f16, tag="oT", name="oT")
        nc.gpsimd.memset(oT[64:80], 0.0)
        nc.vector.tensor_copy(
            oT[0 : D + 1], acc.rearrange("d (b pp) -> d b pp", pp=128)
        )
        o_ti = epi.tile([128, 8, 80], f16, tag="oti", name="oti")
        nc.sync.dma_start_transpose(o_ti, oT)
        rcp = epi.tile([128, 8, 1], f32, tag="rcp", name="rcp")
        nc.vector.reciprocal(rcp, o_ti[:, :, D : D + 1])
        o_f = epi.tile([128, 8, D], f32, tag="of", name="of")
        nc.vector.tensor_mul(
            o_f, o_ti[:, :, 0:D], rcp.broadcast_to([128, 8, D])
        )
        nc.sync.dma_start(
            out=o_d[h].rearrange("(n p) d -> p n d", p=128)[
                :, 8 * p : 8 * p + 8, :
            ],
            in_=o_f,
        )


def _build():
    import concourse.bacc as bacc
    import concourse.mybir as mybir
    import concourse.tile as tile

    f32 = mybir.dt.float32
    f16 = mybir.dt.float16

    nc = bacc.Bacc()
    q_d = nc.dram_tensor("q", [HPC, S, D], f32, kind="ExternalInput")
    k_d = nc.dram_tensor("k", [HPC, S, D], f32, kind="ExternalInput")
    v_d = nc.dram_tensor("v", [HPC, S, D], f32, kind="ExternalInput")
    o_d = nc.dram_tensor("out", [HPC, S, D], f32, kind="ExternalOutput")

    with tile.TileContext(nc) as tc:
        with (
            tc.tile_pool(name="sb", bufs=2) as sb,
            tc.tile_pool(name="exps", bufs=3) as exps,
            tc.tile_pool(name="epi", bufs=2) as epi,
            tc.tile_pool(name="ps", bufs=2, space="PSUM") as ps,
        ):
            for pair in range(HPC // 2):
                # ---- load + stage (two heads per XBAR transpose) ------
                q2 = sb.tile([128, NB, 128], f16, name=f"q2{pair}", tag="q2")
                k2 = sb.tile([128, NB, 128], f16, name=f"k2{pair}", tag="k2")
                vexts = []
                for i in range(2):
                    h = 2 * pair + i
                    q_nat = sb.tile(
                        [128, NB, D], f32, name=f"qn{h}", tag=f"qn{i}"
                    )
                    k_nat = sb.tile(
                        [128, NB, D], f32, name=f"kn{h}", tag=f"kn{i}"
                    )
                    v_stg = sb.tile(
                        [128, NB, D + 1], f32, name=f"vs{h}", tag=f"vs{i}"
                    )
                    nc.sync.dma_start(
                        out=q_nat,
                        in_=q_d[h].rearrange("(n p) d -> p n d", p=128),
                    )
                    nc.sync.dma_start(
                        out=k_nat,
                        in_=k_d[h].rearrange("(n p) d -> p n d", p=128),
                    )
                    nc.sync.dma_start(
                        out=v_stg[:, :, 0:D],
                        in_=v_d[h].rearrange("(n p) d -> p n d", p=128),
                    )
                    nc.gpsimd.memset(v_stg[:, :, D], 1.0)
                    vext = sb.tile(
                        [128, NB, D + 1], f16, name=f"vx{h}", tag=f"vx{i}"
                    )
                    nc.vector.tensor_copy(vext, v_stg)
                    nc.vector.tensor_copy(q2[:, :, i * D : (i + 1) * D], q_nat)
                    nc.vector.tensor_copy(k2[:, :, i * D : (i + 1) * D], k_nat)
                    vexts.append(vext)
                qT2 = sb.tile([128, S], f16, name=f"qT{pair}", tag="qT")
                kT2 = sb.tile([128, S], f16, name=f"kT{pair}", tag="kT")
                nc.sync.dma_start_transpose(
                    qT2.rearrange("d (n p) -> d n p", p=128), q2
                )
                nc.sync.dma_start_transpose(
                    kT2.rearrange("d (n p) -> d n p", p=128), k2
                )

                for i in range(2):
                    _emit_head(
                        nc,
                        mybir,
                        (exps, epi, ps),
                        2 * pair + i,
                        qT2[i * D : (i + 1) * D, :],
                        kT2[i * D : (i + 1) * D, :],
                        vexts[i],
                        o_d,
                    )

    nc.finalize()
    return nc


def _get_nc():
    global _NC
    if _NC is None:
        _install_neff_cache()
        _NC = _build()
    return _NC


def kernel(q, k, v):
    from concourse.bass_utils import run_bass_kernel_spmd

    nc = _get_nc()
    q = np.asarray(q, dtype=np.float32).reshape(B * H, S, D)
    k = np.asarray(k, dtype=np.float32).reshape(B * H, S, D)
    v = np.asarray(v, dtype=np.float32).reshape(B * H, S, D)
    in_maps = [
        {
            "q": q[c * HPC : (c + 1) * HPC],
            "k": k[c * HPC : (c + 1) * HPC],
            "v": v[c * HPC : (c + 1) * HPC],
        }
        for c in range(N_CORES)
    ]
    res = run_bass_kernel_spmd(nc, in_maps, core_ids=list(range(N_CORES)))
    out = np.stack([res.results[c]["out"] for c in range(N_CORES)])
    return out.reshape(B, H, S, D).astype(np.float32)


# revision 11
# speedup vs baseline: 1.0885x; 1.0885x over previous
"""Chunked-causal attention (MemoryEfficientAttention) for Trainium2.

Full inputs q,k,v: [2, 16, 2048, 64] fp32. Causal attention per (batch, head);
chunked reference == plain causal attention. 32 (b,h) slices split 4-per-core
across 8 NeuronCores (data/head parallel, no collectives).

Per-core kernel (4 heads, S=2048, D=64), v2 design:
  - q,k loaded natural [128, 16, 64], cast to fp16 into head-PAIR tiles
    [128, 16, 128] (head a in cols 0:64, head b in 64:128), then transposed
    d-major by the DMA XBAR (InstDmaTransposeAnt): its fold semantics
    out[r % 128, r // 128, p] = in[p, r] land head a's qT on partitions 0:64
    and head b's on 64:128 - zero PE cost, no fixup copies. Head b's matmuls
    use base partition 64 (PE tile position (64, 0)).
  - [V|1] cast to fp16 [128, 16, 65] (ones col -> softmax denominator).
  - per query-pass (1024 wide), per key block jb (128 wide):
      scT[j,i] = kT_jb.T @ qT  (fp16 matmuls -> fp32 PSUM, <=512 col chunks)
      eT = exp(scT/8) fp16     (one ACT instr per (pass, jb))
      diagonal block masked in place on GPSIMD (affine_select, fill 0)
      acc[d|1, i] += vext_jb.T @ eT  (fp16, accumulated in [65,1024] PSUM)
  - epilogue per pass: acc -> oT fp16 [80, 8, 128] (DVE; rows 65:80 pad),
    one XBAR transpose -> o_ti [128, 8, 80] i-major, rcp = 1/denom (DVE),
    out = val*rcp -> f32 (DVE), single DMA store per pass.

PSUM: scT [128,1024] x2 bufs (4 banks) + acc [65,1024] x2 bufs (4 banks) = 8.
Softmax without max-subtraction: scores/8 ~ N(0,1), far from fp32 exp range.
"""

import hashlib
import os

import numpy as np

B, H, S, D = 2, 16, 2048, 64
N_CORES = 8
HPC = (B * H) // N_CORES  # heads per core
NB = S // 128             # 128-row key blocks per head
PASS_W = 1024             # query pass width (2 PSUM banks)
CHUNK = 512               # AV accumulator chunk (1 PSUM bank)

_NC = None


def _install_neff_cache():
    """Content-addressed NEFF cache so repeat runs skip the ~2min walrus compile."""
    import concourse.bass2jax as bass2jax

    real_compile = bass2jax.compile_bir_kernel
    if getattr(bass2jax, "_neff_cache_installed", False):
        return
    cache_dir = os.path.expanduser("~/.cache/bass_neff")
    os.makedirs(cache_dir, exist_ok=True)

    def cached_compile(bir_json, tmpdir, neff_name="file.neff"):
        key = hashlib.sha256(bir_json).hexdigest()[:24]
        path = os.path.join(cache_dir, f"{key}.neff")
        if os.path.exists(path):
            dst = os.path.join(tmpdir, neff_name)
            with open(path, "rb") as f_in, open(dst, "wb") as f_out:
                f_out.write(f_in.read())
            return dst
        neff = real_compile(bir_json, tmpdir, neff_name)
        with open(neff, "rb") as f_in, open(path + ".tmp", "wb") as f_out:
            f_out.write(f_in.read())
        os.replace(path + ".tmp", path)
        return neff

    bass2jax.compile_bir_kernel = cached_compile
    bass2jax._neff_cache_installed = True


def _emit_head(nc, mybir, pools, h, qT, kT, vext, o_d):
    """One head's QK/exp/AV/epilogue. qT/kT [64, S] may start at partition 64."""
    f32 = mybir.dt.float32
    f16 = mybir.dt.float16
    Exp = mybir.ActivationFunctionType.Exp
    exps, epi, ps = pools

    for p in range(2):
        ilo_p, ihi_p = p * PASS_W, (p + 1) * PASS_W
        acc = ps.tile([D + 1, PASS_W], f32, tag="acc", bufs=1, name=f"ac{h}{p}")
        n_jb = 8 * p + 8
        eTs = {}

        def emit_av(jb):
            i_lo = max(jb * 128, ilo_p)
            for c in range(2):
                g = 2 * p + c
                ch_lo, ch_hi = ilo_p + c * CHUNK, ilo_p + (c + 1) * CHUNK
                if ch_hi <= i_lo:
                    continue
                a_lo = max(i_lo, ch_lo)
                nc.tensor.matmul(
                    acc[:, a_lo - ilo_p : ch_hi - ilo_p],
                    vext[:, jb, :],
                    eTs[jb][:, a_lo - i_lo : ch_hi - i_lo],
                    start=(jb == 0),
                    stop=(jb == 4 * g + 3),
                )

        for jb in range(n_jb):
            j0 = jb * 128
            i_lo = max(j0, ilo_p)
            w = ihi_p - i_lo
            scT = ps.tile([128, PASS_W], f32, tag="sc", bufs=3, name="scT")
            for c0 in range(0, w, CHUNK):
                c1 = min(c0 + CHUNK, w)
                nc.tensor.matmul(
                    scT[:, c0:c1],
                    kT[:, j0 : j0 + 128],
                    qT[:, i_lo + c0 : i_lo + c1],
                    start=True,
                    stop=True,
                )
            eT = exps.tile([128, PASS_W], f16, tag="eT", name="eT")
            eTs[jb] = eT
            nc.scalar.activation(
                eT[:, 0:w], scT[:, 0:w], Exp, scale=float(D) ** -0.5
            )
            if j0 >= ilo_p:
                # diagonal block: keep j <= i (iota = i - j >= 0)
                nc.gpsimd.affine_select(
                    out=eT[:, 0:128],
                    in_=eT[:, 0:128],
                    compare_op=mybir.AluOpType.is_ge,
                    fill=0.0,
                    pattern=[[1, 128]],
                    channel_multiplier=-1,
                )
            # lag the AV two iterations behind QK so the in-order PE queue
            # never head-of-line blocks on an exp that isn't ready yet
            if jb >= 2:
                emit_av(jb - 2)
        emit_av(n_jb - 2)
        emit_av(n_jb - 1)

        # ---- epilogue: normalize + transpose via XBAR -------------
        # oT [80, (blk, pp)] fp16; one XBAR instr yields o_ti[pp, blk, d].
        # Rows 65:80 are pad (XBAR wants a multiple of 16 rows).
        oT = epi.tile([80, 8, 128], f16, tag="oT", name="oT")
        nc.gpsimd.memset(oT[64:80], 0.0)
        nc.vector.tensor_copy(
            oT[0 : D + 1], acc.rearrange("d (b pp) -> d b pp", pp=128)
        )
        o_ti = epi.tile([128, 8, 80], f16, tag="oti", name="oti")
        nc.sync.dma_start_transpose(o_ti, oT)
        rcp = epi.tile([128, 8, 1], f32, tag="rcp", name="rcp")
        nc.vector.reciprocal(rcp, o_ti[:, :, D : D + 1])
        o_f = epi.tile([128, 8, D], f32, tag="of", name="of")
        nc.vector.tensor_mul(
            o_f, o_ti[:, :, 0:D], rcp.broadcast_to([128, 8, D])
        )
        nc.sync.dma_start(
            out=o_d[h].rearrange("(n p) d -> p n d", p=128)[
                :, 8 * p : 8 * p + 8, :
            ],
            in_=o_f,
        )


def _build():
    import concourse.bacc as bacc
    import concourse.mybir as mybir
    import concourse.tile as tile

    f32 = mybir.dt.float32
    f16 = mybir.dt.float16

    nc = bacc.Bacc()
    q_d = nc.dram_tensor("q", [HPC, S, D], f32, kind="ExternalInput")
    k_d = nc.dram_tensor("k", [HPC, S, D], f32, kind="ExternalInput")
    v_d = nc.dram_tensor("v", [HPC, S, D], f32, kind="ExternalInput")
    o_d = nc.dram_tensor("out", [HPC, S, D], f32, kind="ExternalOutput")

    with tile.TileContext(nc) as tc:
        with (
            tc.tile_pool(name="sb", bufs=2) as sb,
            tc.tile_pool(name="exps", bufs=4) as exps,
            tc.tile_pool(name="epi", bufs=2) as epi,
            tc.tile_pool(name="ps", bufs=2, space="PSUM") as ps,
        ):
            for pair in range(HPC // 2):
                # ---- load + stage (two heads per XBAR transpose) ------
                q2 = sb.tile([128, NB, 128], f16, name=f"q2{pair}", tag="q2")
                k2 = sb.tile([128, NB, 128], f16, name=f"k2{pair}", tag="k2")
                vexts = []
                for i in range(2):
                    h = 2 * pair + i
                    q_nat = sb.tile(
                        [128, NB, D], f32, name=f"qn{h}", tag=f"qn{i}"
                    )
                    k_nat = sb.tile(
                        [128, NB, D], f32, name=f"kn{h}", tag=f"kn{i}"
                    )
                    v_stg = sb.tile(
                        [128, NB, D + 1], f32, name=f"vs{h}", tag=f"vs{i}"
                    )
                    nc.sync.dma_start(
                        out=q_nat,
                        in_=q_d[h].rearrange("(n p) d -> p n d", p=128),
                    )
                    nc.sync.dma_start(
                        out=k_nat,
                        in_=k_d[h].rearrange("(n p) d -> p n d", p=128),
                    )
                    nc.sync.dma_start(
                        out=v_stg[:, :, 0:D],
                        in_=v_d[h].rearrange("(n p) d -> p n d", p=128),
                    )
                    nc.gpsimd.memset(v_stg[:, :, D], 1.0)
                    vext = sb.tile(
                        [128, NB, D + 1], f16, name=f"vx{h}", tag=f"vx{i}"
                    )
                    nc.vector.tensor_copy(vext, v_stg)
                    nc.vector.tensor_copy(q2[:, :, i * D : (i + 1) * D], q_nat)
                    nc.vector.tensor_copy(k2[:, :, i * D : (i + 1) * D], k_nat)
                    vexts.append(vext)
                qT2 = sb.tile([128, S], f16, name=f"qT{pair}", tag="qT")
                kT2 = sb.tile([128, S], f16, name=f"kT{pair}", tag="kT")
                nc.sync.dma_start_transpose(
                    qT2.rearrange("d (n p) -> d n p", p=128), q2
                )
                nc.sync.dma_start_transpose(
                    kT2.rearrange("d (n p) -> d n p", p=128), k2
                )

                for i in range(2):
                    _emit_head(
                        nc,
                        mybir,
                        (exps, epi, ps),
                        2 * pair + i,
                        qT2[i * D : (i + 1) * D, :],
                        kT2[i * D : (i + 1) * D, :],
                        vexts[i],
                        o_d,
                    )

    nc.finalize()
    return nc


def _get_nc():
    global _NC
    if _NC is None:
        _install_neff_cache()
        _NC = _build()
    return _NC


def kernel(q, k, v):
    from concourse.bass_utils import run_bass_kernel_spmd

    nc = _get_nc()
    q = np.asarray(q, dtype=np.float32).reshape(B * H, S, D)
    k = np.asarray(k, dtype=np.float32).reshape(B * H, S, D)
    v = np.asarray(v, dtype=np.float32).reshape(B * H, S, D)
    in_maps = [
        {
            "q": q[c * HPC : (c + 1) * HPC],
            "k": k[c * HPC : (c + 1) * HPC],
            "v": v[c * HPC : (c + 1) * HPC],
        }
        for c in range(N_CORES)
    ]
    res = run_bass_kernel_spmd(nc, in_maps, core_ids=list(range(N_CORES)))
    out = np.stack([res.results[c]["out"] for c in range(N_CORES)])
    return out.reshape(B, H, S, D).astype(np.float32)
